# revision 7
# baseline (speedup 1.0000x reference)
"""Trainium2 Bass kernel for per-query bilinear-interpolated 3x3 affine
transform (embedding-lookup style), data-parallel across 8 NeuronCores.

Math per query n:
    iu = u[n]*400, jv = v[n]*400
    i1 = floor(iu), j1 = floor(jv); ir = iu-i1, jr = jv-j1
    texels (i1,j1),(i1+1,j1),(i1,j1+1),(i1+1,j1+1)  (wrap mod 400)
    W = bilinear-mix of per-texel 3x3 matrices; B = same for 1x3 biases
    out[n] = x[n] @ W + B

Host-side prep: build a "patch table" [4*400*400, 48] where each row
holds the full 2x2 texel neighborhood (4 x (3x3 matrix + bias)) with
wraparound baked in.  On device each query then needs ONE contiguous
48-float gather (indirect DMA) + pure elementwise math.
"""

import sys

if "/opt/trn_rl_repo" not in sys.path:
    sys.path.insert(0, "/opt/trn_rl_repo")

import numpy as np

U = 400
V = 400
M = 4
N_CORES = 8
P = 128

# Tunables: per-core shard = P*K*T queries.
K = 200  # queries per partition per macro-tile
T = 20  # macro-tiles per core
N_SHARD = P * K * T  # 512000
N_PAD = N_CORES * N_SHARD  # 4_096_000 (inputs padded up to this)

REC = 48  # floats per patch record


def _build_patch_table(m_param: np.ndarray, b_param: np.ndarray) -> np.ndarray:
    """[M, U*V, 3, 3] + [M, U*V, 1, 3] -> [M*U*V, 48] patch records.

    Record layout: [T(i,j), T(i+1,j), T(i,j+1), T(i+1,j+1)] where each
    texel block is 12 floats: the 3x3 matrix row-major then the bias.
    """
    mb = np.concatenate(
        [
            np.asarray(m_param, np.float32).reshape(M, U, V, 9),
            np.asarray(b_param, np.float32).reshape(M, U, V, 3),
        ],
        axis=-1,
    )  # [M, U, V, 12]
    r10 = np.roll(mb, -1, axis=1)
    r01 = np.roll(mb, -1, axis=2)
    r11 = np.roll(r10, -1, axis=2)
    patch = np.concatenate([mb, r10, r01, r11], axis=-1)  # [M, U, V, 48]
    return np.ascontiguousarray(patch.reshape(M * U * V, REC))


def _split_multi_waits(nc, max_waits: int = 1):
    """This walrus build rejects instructions carrying more than one sync
    wait. Hoist extra waits onto same-engine no-ops inserted just before
    the offending instruction (sequencer executes them in order, so the
    semantics are identical)."""
    from concourse import mybir

    for fn in nc.m.functions:
        for bb in fn.blocks:
            new_insts = []
            changed = False
            for inst in bb.instructions:
                si = inst.sync_info
                if si is not None and si.on_wait and len(si.on_wait) > max_waits:
                    waits = list(si.on_wait)
                    keep = waits[-max_waits:]
                    hoist = waits[:-max_waits]
                    for i in range(0, len(hoist), max_waits):
                        nop = mybir.InstNoOp(
                            name=nc.get_next_instruction_name(), ins=[], outs=[]
                        )
                        nop.engine = inst.engine
                        nop.sync_info = mybir.SyncInfo(
                            on_wait=hoist[i : i + max_waits], on_update=[]
                        )
                        nc.register_instruction(nop)
                        new_insts.append(nop)
                    si.on_wait = keep
                    changed = True
                new_insts.append(inst)
            if changed:
                bb.instructions = new_insts


def build_program(n_shard: int = N_SHARD, k: int = K):
    """Build the single-core SPMD Bass program (same graph on all cores)."""
    from concourse import bass, mybir
    import concourse.tile as tile

    assert n_shard % (P * k) == 0
    n_tiles = n_shard // (P * k)
    f32 = mybir.dt.float32
    i32 = mybir.dt.int32
    Alu = mybir.AluOpType

    nc = bass.Bass()
    x_ext = nc.declare_dram_parameter("xq", [n_shard, 3], f32, isOutput=False)
    u_ext = nc.declare_dram_parameter("uq", [n_shard], f32, isOutput=False)
    v_ext = nc.declare_dram_parameter("vq", [n_shard], f32, isOutput=False)
    m_ext = nc.declare_dram_parameter("mq", [n_shard], i32, isOutput=False)
    tbl_ext = nc.declare_dram_parameter("tbl", [M * U * V, REC], f32, isOutput=False)
    out_ext = nc.declare_dram_parameter("out", [n_shard, 3], f32, isOutput=True)

    import os
    repeat = int(os.environ.get("K1_REPEAT", "1"))
    with tile.TileContext(nc) as tc:
        with (
            tc.tile_pool(name="io", bufs=3) as io_pool,
            tc.tile_pool(name="rec", bufs=2) as rec_pool,
            tc.tile_pool(name="sc", bufs=2) as sc_pool,
        ):
            for t in range(repeat * n_tiles):
                n0 = (t % n_tiles) * P * k
                n1 = n0 + P * k
                u_d = u_ext[n0:n1].rearrange("(p k) -> p k", p=P)
                v_d = v_ext[n0:n1].rearrange("(p k) -> p k", p=P)
                m_d = m_ext[n0:n1].rearrange("(p k) -> p k", p=P)
                x_d = x_ext[n0:n1, :].rearrange("(p k) c -> p k c", p=P)
                o_d = out_ext[n0:n1, :].rearrange("(p k) c -> p k c", p=P)

                ut = io_pool.tile([P, k], f32, tag="ut")
                vt = io_pool.tile([P, k], f32, tag="vt")
                mt = io_pool.tile([P, k], i32, tag="mt")
                xt = io_pool.tile([P, k, 3], f32, tag="xt")
                nc.sync.dma_start(out=ut[:], in_=u_d)
                nc.sync.dma_start(out=vt[:], in_=v_d)
                nc.sync.dma_start(out=mt[:], in_=m_d)
                nc.sync.dma_start(out=xt[:], in_=x_d)

                # per-query scalars (all [P, k] f32)
                iu = sc_pool.tile([P, k], f32, tag="iu")
                jv = sc_pool.tile([P, k], f32, tag="jv")
                ir = sc_pool.tile([P, k], f32, tag="ir")
                jr = sc_pool.tile([P, k], f32, tag="jr")
                i1f = sc_pool.tile([P, k], f32, tag="i1f")
                j1f = sc_pool.tile([P, k], f32, tag="j1f")
                ii = sc_pool.tile([P, k], i32, tag="ii")
                jj = sc_pool.tile([P, k], i32, tag="jj")
                wi0 = sc_pool.tile([P, k], f32, tag="wi0")
                wj0 = sc_pool.tile([P, k], f32, tag="wj0")
                w11 = sc_pool.tile([P, k], f32, tag="w11")
                w21 = sc_pool.tile([P, k], f32, tag="w21")
                w12 = sc_pool.tile([P, k], f32, tag="w12")
                w22 = sc_pool.tile([P, k], f32, tag="w22")
                idxf = sc_pool.tile([P, k], f32, tag="idxf")
                idxi = sc_pool.tile([P, k], i32, tag="idxi")

                # floor(iu): HW f32->i32 cast rounds to nearest-even, so
                # round-trip then subtract (rounded > iu) to get the floor.
                nc.vector.tensor_scalar_mul(out=iu[:], in0=ut[:], scalar1=float(U))
                nc.vector.tensor_copy(out=ii[:], in_=iu[:])
                nc.vector.tensor_copy(out=i1f[:], in_=ii[:])
                nc.vector.tensor_tensor(out=ir[:], in0=i1f[:], in1=iu[:], op=Alu.is_gt)
                nc.vector.tensor_tensor(out=i1f[:], in0=i1f[:], in1=ir[:], op=Alu.subtract)
                nc.vector.tensor_tensor(out=ir[:], in0=iu[:], in1=i1f[:], op=Alu.subtract)

                nc.vector.tensor_scalar_mul(out=jv[:], in0=vt[:], scalar1=float(V))
                nc.vector.tensor_copy(out=jj[:], in_=jv[:])
                nc.vector.tensor_copy(out=j1f[:], in_=jj[:])
                nc.vector.tensor_tensor(out=jr[:], in0=j1f[:], in1=jv[:], op=Alu.is_gt)
                nc.vector.tensor_tensor(out=j1f[:], in0=j1f[:], in1=jr[:], op=Alu.subtract)
                nc.vector.tensor_tensor(out=jr[:], in0=jv[:], in1=j1f[:], op=Alu.subtract)

                # wi0 = 1-ir, wj0 = 1-jr
                nc.vector.tensor_scalar(
                    out=wi0[:], in0=ir[:], scalar1=-1.0, scalar2=1.0,
                    op0=Alu.mult, op1=Alu.add,
                )
                nc.vector.tensor_scalar(
                    out=wj0[:], in0=jr[:], scalar1=-1.0, scalar2=1.0,
                    op0=Alu.mult, op1=Alu.add,
                )
                nc.vector.tensor_tensor(out=w11[:], in0=wi0[:], in1=wj0[:], op=Alu.mult)
                nc.vector.tensor_tensor(out=w21[:], in0=ir[:], in1=wj0[:], op=Alu.mult)
                nc.vector.tensor_tensor(out=w12[:], in0=wi0[:], in1=jr[:], op=Alu.mult)
                nc.vector.tensor_tensor(out=w22[:], in0=ir[:], in1=jr[:], op=Alu.mult)

                # idx = m*160000 + i1*400 + j1  (exact in f32; int inputs are
                # converted to f32 inside the DVE ALU)
                nc.vector.scalar_tensor_tensor(
                    out=idxf[:], in0=i1f[:], scalar=float(V), in1=j1f[:],
                    op0=Alu.mult, op1=Alu.add,
                )
                nc.vector.scalar_tensor_tensor(
                    out=idxf[:], in0=mt[:], scalar=float(U * V), in1=idxf[:],
                    op0=Alu.mult, op1=Alu.add,
                )
                nc.vector.tensor_copy(out=idxi[:], in_=idxf[:])

                # gather: one 48-float record per query; the HW indirect DMA
                # consumes ONE index per partition per instruction, so issue
                # k instructions of 128 rows each
                rec = rec_pool.tile([P, k, REC], f32, tag="rec")
                for kk in range(k):
                    nc.gpsimd.indirect_dma_start(
                        out=rec[:][:, kk, :],
                        out_offset=None,
                        in_=tbl_ext[:],
                        in_offset=bass.IndirectOffsetOnAxis(
                            ap=idxi[:][:, kk : kk + 1], axis=0
                        ),
                    )

                r = rec[:]
                ab = [r[:, :, 12 * a : 12 * (a + 1)] for a in range(4)]
                wb = [
                    w[:].to_broadcast([P, k, 12]) for w in (w11, w21, w12, w22)
                ]
                # weighted mix of the 4 texel blocks (in place)
                for a in range(4):
                    nc.vector.tensor_tensor(out=ab[a], in0=ab[a], in1=wb[a], op=Alu.mult)
                nc.vector.tensor_tensor(out=ab[0], in0=ab[0], in1=ab[1], op=Alu.add)
                nc.vector.tensor_tensor(out=ab[2], in0=ab[2], in1=ab[3], op=Alu.add)
                nc.vector.tensor_tensor(out=ab[0], in0=ab[0], in1=ab[2], op=Alu.add)

                # apply x: out_j = sum_i x_i*A[3i+j] + A[9+j]
                A = r
                ot = io_pool.tile([P, k, 3], f32, tag="ot")
                tmp = io_pool.tile([P, k, 3], f32, tag="tmp")
                xv = xt[:]
                x0 = xv[:, :, 0:1].to_broadcast([P, k, 3])
                x1 = xv[:, :, 1:2].to_broadcast([P, k, 3])
                x2 = xv[:, :, 2:3].to_broadcast([P, k, 3])
                nc.vector.tensor_tensor(out=ot[:], in0=A[:, :, 0:3], in1=x0, op=Alu.mult)
                nc.vector.tensor_tensor(out=tmp[:], in0=A[:, :, 3:6], in1=x1, op=Alu.mult)
                nc.vector.tensor_tensor(out=ot[:], in0=ot[:], in1=tmp[:], op=Alu.add)
                nc.vector.tensor_tensor(out=tmp[:], in0=A[:, :, 6:9], in1=x2, op=Alu.mult)
                nc.vector.tensor_tensor(out=ot[:], in0=ot[:], in1=tmp[:], op=Alu.add)
                nc.vector.tensor_tensor(
                    out=ot[:], in0=ot[:], in1=A[:, :, 9:12], op=Alu.add
                )

                nc.sync.dma_start(out=o_d, in_=ot[:])

    _split_multi_waits(nc)
    return nc


_prog_cache: dict = {}


def _get_program(n_shard: int, k: int):
    key = (n_shard, k)
    if key not in _prog_cache:
        _prog_cache[key] = build_program(n_shard, k)
    return _prog_cache[key]


def _run(x, m, u, v, m_param, b_param, trace=False):
    from concourse.bass_utils import run_bass_kernel_spmd

    n = x.shape[0]
    tbl = _build_patch_table(m_param, b_param)

    def pad(a):
        if a.shape[0] == N_PAD:
            return np.ascontiguousarray(a)
        p = np.zeros((N_PAD, *a.shape[1:]), a.dtype)
        p[: a.shape[0]] = a
        return p

    xp = pad(np.asarray(x, np.float32))
    up = pad(np.asarray(u, np.float32))
    vp = pad(np.asarray(v, np.float32))
    mp = pad(np.asarray(m, np.int32))

    nc = _get_program(N_SHARD, K)
    in_maps = []
    for c in range(N_CORES):
        s = slice(c * N_SHARD, (c + 1) * N_SHARD)
        in_maps.append(
            {"xq": xp[s], "uq": up[s], "vq": vp[s], "mq": mp[s], "tbl": tbl}
        )
    res = run_bass_kernel_spmd(
        nc, in_maps, core_ids=list(range(N_CORES)), trace=trace
    )
    out = np.concatenate([res.results[c]["out"] for c in range(N_CORES)], axis=0)
    return out[:n], res


def kernel(x, m, u, v, m_param, b_param):
    out, _ = _run(x, m, u, v, m_param, b_param, trace=False)
    return out



